# revision 2
# baseline (speedup 1.0000x reference)
"""Trainium2 Bass kernel for a quantized KAN layer (B-spline MLP).

  out[b,o] = x @ base_weight.T + einsum('bic,oic->bo', bspline_basis(x), round(32*w)/32)

Strategy (8 NeuronCores, contraction/i-sharded), v3:
  - Cubic B-splines on a uniform grid reproduce constants and linears
    exactly on [-1, 1]:  sum_c B_c(x) = 1  and  sum_c gamma_c B_c(x) = x
    with Greville abscissae gamma_c = (c-1)*h - 1. So the base matmul
    folds into the spline weights (v_c = q_c + gamma_c*bw) and channel 7
    folds into a per-output bias (w''_c = v_c - v_7, bias_o = sum_i v_7).
    The contraction shrinks from 9 to 7 channels: 14 k-tiles per core.
  - v3: quantization + folding runs on the HOST in f32 (bit-identical
    RNE math to the old on-device prologue) and the kernel receives the
    folded weights pre-packed as fp16 [128, 14, 2048] per core. This
    cuts the on-device weight prologue from ~34 MB of f32 DMA + a
    quantize chain (~720 us serial) to 7.3 MB of fp16 DMA split across
    both HWDGE queues (SP + ACT) so it streams in parallel.
  - Output is computed transposed ([out, batch]) so bias_o is a
    per-partition bias applied for free in the PSUM->SBUF copy.
  - The basis chain runs in fp16 spread over ACT/DVE/Pool so it hides
    under the matmul stream: t = x/h - (c - 1 - 1/h); a = s2*|t| (ACT
    Abs with folded scale); r2 = relu(2*s2 - a), r1 = relu(s1 - s1/s2*a)
    (DVE tensor_scalar 4x); cubes on DVE/ACT; final sub on Pool.
  - Matmuls are fp16 x fp16 -> f32 PSUM, one 512-col moving matmul per
    stationary, k-inner order; the main loop is PE-bound at ~216 ns/MM.
  - Host sums the 8 partial [2048, 4096] outputs and transposes.
"""

import numpy as np

B, IN, OUT = 4096, 2048, 2048
NCORES = 8
ISH = IN // NCORES          # 256 input features per core
P = 128
NT = ISH // P               # 2 i-tiles per core
NCH = 8                     # spline channels in the reference
NCH7 = 7                    # folded channels on device
KT = NT * NCH7              # 14 k-tiles
BCH = 512                   # batch chunk
NBC = B // BCH              # 8
NOB = OUT // P              # 16 output blocks
S2 = float((1.0 / 6.0) ** (1.0 / 3.0))
S1 = float((4.0 / 6.0) ** (1.0 / 3.0))

_BUILT = {}


def _build(h, repeat=1):
    from concourse import bacc, bass, mybir, tile

    f32 = mybir.dt.float32
    fp16 = mybir.dt.float16
    AF = mybir.ActivationFunctionType
    ALU = mybir.AluOpType

    nc = bacc.Bacc("TRN2", target_bir_lowering=False, debug=False)

    xt = nc.dram_tensor("xt", [ISH, B], f32, kind="ExternalInput")
    # Host-folded fp16 weights: [p][k = t*7+c][o]
    wfh = nc.dram_tensor("wfh", [P, KT * OUT], fp16, kind="ExternalInput")
    # Per-output bias, laid out [p][ob]
    biasd = nc.dram_tensor("biasd", [P, NOB], f32, kind="ExternalInput")
    outp = nc.dram_tensor("outp", [OUT, B], f32, kind="ExternalOutput")

    with tile.TileContext(nc) as tc:
        with tc.tile_pool(name="const", bufs=1) as cpool:
            # Resident folded weights: [128, 14, 2048] fp16 (56KB/partition).
            wf = cpool.tile([P, KT, OUT], fp16)
            # Split the 7.3MB weight load across both HWDGE queues.
            half = (KT // 2) * OUT
            nc.sync.dma_start(wf[:, 0:KT // 2, :], wfh[:, 0:half])
            nc.scalar.dma_start(wf[:, KT // 2:KT, :], wfh[:, half:KT * OUT])
            bias_sb = cpool.tile([P, NOB], f32)
            nc.sync.dma_start(bias_sb[:], biasd[:, :])
            # Channel centers in h units: (c-1) - 1/h, exact in fp16 for h=.4
            gt = cpool.tile([P, NCH7, BCH], fp16)
            for c in range(NCH7):
                nc.vector.memset(gt[:, c, :], float(c - 1) - 1.0 / h)

            # ---- main loop ----
            with (
                tc.tile_pool(name="xin", bufs=2) as xpool,
                tc.tile_pool(name="tmp", bufs=1) as tpool,
                tc.tile_pool(name="bas", bufs=3) as bpool,
                tc.tile_pool(name="outsb", bufs=4) as opool,
                tc.tile_pool(name="psum", bufs=8,
                             space=bass.MemorySpace.PSUM) as ppool,
            ):
                for bc in [c for _ in range(repeat) for c in range(NBC)]:
                    bas = []
                    for t in range(NT):
                        xc = xpool.tile([P, BCH], f32, tag=f"xc{t}")
                        nc.sync.dma_start(
                            xc[:], xt[t * P:(t + 1) * P,
                                      bc * BCH:(bc + 1) * BCH])
                        xh = xpool.tile([P, BCH], fp16, tag=f"xh{t}")
                        nc.vector.tensor_scalar(xh[:], xc[:], 1.0 / h, None,
                                                ALU.mult)
                        sh3 = [P, NCH7, BCH]
                        xb = xh[:].unsqueeze(1).broadcast_to(sh3)
                        t8 = tpool.tile(sh3, fp16, tag=f"t8{t}")
                        nc.vector.tensor_sub(t8[:], xb, gt[:])
                        # t8 := a = s2*|t|  (ACT abs with folded scale)
                        nc.scalar.activation(t8[:], t8[:], AF.Abs, scale=S2)
                        r2 = tpool.tile(sh3, fp16, tag=f"r2{t}")
                        nc.vector.tensor_scalar(r2[:], t8[:], -1.0, 2.0 * S2,
                                                ALU.mult, ALU.add)
                        nc.vector.tensor_scalar_max(r2[:], r2[:], 0.0)
                        v = tpool.tile(sh3, fp16, tag=f"v{t}")
                        nc.vector.tensor_scalar(v[:], t8[:], -S1 / S2, S1,
                                                ALU.mult, ALU.add)
                        nc.vector.tensor_scalar_max(v[:], v[:], 0.0)  # := r1
                        q2 = tpool.tile(sh3, fp16, tag=f"q2{t}")
                        nc.vector.tensor_mul(q2[:], r2[:], r2[:])
                        q1 = tpool.tile(sh3, fp16, tag=f"q1{t}")
                        nc.scalar.activation(q1[:], v[:], AF.Square)
                        nc.vector.tensor_mul(q2[:], q2[:], r2[:])  # := r2^3
                        nc.vector.tensor_mul(q1[:], q1[:], v[:])   # := r1^3
                        bt_ = bpool.tile(sh3, fp16, tag=f"bas{t}")
                        nc.gpsimd.tensor_sub(bt_[:], q2[:], q1[:])
                        bas.append(bt_)

                    for ob in range(NOB):
                        ps = ppool.tile([P, BCH], f32, tag="ps")
                        k = 0
                        for t in range(NT):
                            for c in range(NCH7):
                                nc.tensor.matmul(
                                    ps[:],
                                    wf[:, t * NCH7 + c, ob * P:(ob + 1) * P],
                                    bas[t][:, c, :],
                                    start=(k == 0), stop=(k == KT - 1))
                                k += 1
                        osb = opool.tile([P, BCH], f32, tag="osb")
                        nc.scalar.activation(osb[:], ps[:], AF.Identity,
                                             bias=bias_sb[:, ob:ob + 1],
                                             scale=1.0)
                        nc.sync.dma_start(
                            outp[ob * P:(ob + 1) * P,
                                 bc * BCH:(bc + 1) * BCH], osb[:])

    nc.compile()
    return nc


def _stage(x, base_weight, spline_weight, grid):
    """Per-core staging: quantize + fold on host in f32 (bit-identical to
    the old on-device prologue: same RNE rounding, same op order), pack
    weights fp16."""
    h = np.float32(grid[0, 1] - grid[0, 0])
    gam7 = np.float32((NCH - 2) * h - 1.0)
    in_maps = []
    # round(32w)/32 in f32, RNE — matches the device magic-number round.
    q_all = (np.round(spline_weight.astype(np.float32) * np.float32(32.0))
             * np.float32(1.0 / 32.0)).astype(np.float32)
    for j in range(NCORES):
        sh = slice(j * ISH, (j + 1) * ISH)
        xt = np.ascontiguousarray(x[:, sh].T)
        q = q_all[:, sh, :]                       # [OUT, 256, 8] f32
        bw = base_weight[:, sh].astype(np.float32)  # [OUT, 256]
        q7 = q[:, :, NCH - 1]
        v7 = q7 + gam7 * bw                        # [OUT, 256]
        bias = v7.sum(axis=1, dtype=np.float32)    # [OUT]
        # folded channels c=0..6: (q_c - q_7) + (c-7)*h*bw
        wfold = np.empty((OUT, ISH, NCH7), dtype=np.float32)
        for c in range(NCH7):
            wfold[:, :, c] = (q[:, :, c] - q7) + np.float32((c - 7) * h) * bw
        # layout [p, t*7+c, o]: i = t*128+p
        wfh = wfold.reshape(OUT, NT, P, NCH7).transpose(2, 1, 3, 0)
        wfh = np.ascontiguousarray(
            wfh.reshape(P, KT * OUT).astype(np.float16))
        biasd = np.ascontiguousarray(bias.reshape(NOB, P).T)
        in_maps.append({"xt": xt, "wfh": wfh, "biasd": biasd})
    return in_maps


def kernel(x, base_weight, spline_weight, grid, _profile=None):
    from concourse import bass_utils

    x = np.asarray(x, dtype=np.float32)
    base_weight = np.asarray(base_weight, dtype=np.float32)
    spline_weight = np.asarray(spline_weight, dtype=np.float32)
    grid = np.asarray(grid, dtype=np.float32)

    h = float(grid[0, 1] - grid[0, 0])
    key = round(h, 9)
    if key not in _BUILT:
        _BUILT[key] = _build(h)
    nc = _BUILT[key]

    in_maps = _stage(x, base_weight, spline_weight, grid)
    kw = {}
    if _profile is not None:
        kw = _profile
    res = bass_utils.run_bass_kernel_spmd(
        nc, in_maps, core_ids=list(range(NCORES)), **kw)

    out_T = np.zeros((OUT, B), dtype=np.float32)
    for om in res.results:
        out_T += np.asarray(om["outp"], dtype=np.float32)
    if _profile is not None:
        kernel._last_result = res
    return np.ascontiguousarray(out_T.T)


# revision 3
# speedup vs baseline: 2.6449x; 2.6449x over previous
"""Trainium2 Bass kernel for a quantized KAN layer (B-spline MLP).

  out[b,o] = x @ base_weight.T + einsum('bic,oic->bo', bspline_basis(x), round(32*w)/32)

Strategy (8 NeuronCores, contraction/i-sharded), v3:
  - Cubic B-splines on a uniform grid reproduce constants and linears
    exactly on [-1, 1]:  sum_c B_c(x) = 1  and  sum_c gamma_c B_c(x) = x
    with Greville abscissae gamma_c = (c-1)*h - 1. So the base matmul
    folds into the spline weights (v_c = q_c + gamma_c*bw) and channel 7
    folds into a per-output bias (w''_c = v_c - v_7, bias_o = sum_i v_7).
    The contraction shrinks from 9 to 7 channels: 14 k-tiles per core.
  - v3: quantization + folding runs on the HOST in f32 (bit-identical
    RNE math to the old on-device prologue) and the kernel receives the
    folded weights pre-packed as fp16 [128, 14, 2048] per core. This
    cuts the on-device weight prologue from ~34 MB of f32 DMA + a
    quantize chain to 7.3 MB of fp16 DMA, issued as 14 per-k-tile
    transfers alternating across both HWDGE queues (SP + ACT) so the
    k-inner matmul stream can start as soon as the first tiles land.
    Chunk-0 x tiles are prefetched ahead of the weight stream so the
    basis chain overlaps the weight load.
  - Output is computed transposed ([out, batch]) so bias_o is a
    per-partition bias applied for free in the PSUM->SBUF copy.
  - The basis chain runs in fp16 spread over ACT/DVE/Pool so it hides
    under the matmul stream: t = x/h - (c - 1 - 1/h); a = s2*|t| (ACT
    Abs with folded scale); r2 = relu(2*s2 - a), r1 = relu(s1 - s1/s2*a)
    (DVE tensor_scalar 4x); cubes on DVE/ACT; final sub on Pool.
  - Matmuls are fp16 x fp16 -> f32 PSUM, one 512-col moving matmul per
    stationary, k-inner order; the main loop is PE-bound at ~216 ns/MM.
  - Host sums the 8 partial [2048, 4096] outputs and transposes.
"""

import numpy as np

B, IN, OUT = 4096, 2048, 2048
NCORES = 8
ISH = IN // NCORES          # 256 input features per core
P = 128
NT = ISH // P               # 2 i-tiles per core
NCH = 8                     # spline channels in the reference
NCH7 = 7                    # folded channels on device
KT = NT * NCH7              # 14 k-tiles
BCH = 512                   # batch chunk
NBC = B // BCH              # 8
NOB = OUT // P              # 16 output blocks
S2 = float((1.0 / 6.0) ** (1.0 / 3.0))
S1 = float((4.0 / 6.0) ** (1.0 / 3.0))

_BUILT = {}


def _build(h, repeat=1, wrep=1):
    from concourse import bacc, bass, mybir, tile

    f32 = mybir.dt.float32
    fp16 = mybir.dt.float16
    AF = mybir.ActivationFunctionType
    ALU = mybir.AluOpType

    nc = bacc.Bacc("TRN2", target_bir_lowering=False, debug=False)

    xt = nc.dram_tensor("xt", [ISH, B], f32, kind="ExternalInput")
    # Host-folded fp16 weights: [p][k = t*7+c][o]
    wfh = nc.dram_tensor("wfh", [P, KT * OUT], fp16, kind="ExternalInput")
    # Per-output bias, laid out [p][ob]
    biasd = nc.dram_tensor("biasd", [P, NOB], f32, kind="ExternalInput")
    outp = nc.dram_tensor("outp", [OUT, B], f32, kind="ExternalOutput")

    with tile.TileContext(nc) as tc:
        with (
            tc.tile_pool(name="const", bufs=1) as cpool,
            tc.tile_pool(name="xin", bufs=2) as xpool,
            tc.tile_pool(name="tmp", bufs=1) as tpool,
            tc.tile_pool(name="bas", bufs=3) as bpool,
            tc.tile_pool(name="outsb", bufs=4) as opool,
            tc.tile_pool(name="psum", bufs=8,
                         space=bass.MemorySpace.PSUM) as ppool,
        ):
            # Prefetch chunk-0 x ahead of the weight stream (SP queue).
            pref = {}
            for t in range(NT):
                xc = xpool.tile([P, BCH], f32, tag=f"xc{t}")
                nc.sync.dma_start(xc[:], xt[t * P:(t + 1) * P, 0:BCH])
                pref[t] = xc

            # Resident folded weights: [128, 14, 2048] fp16 (56KB/partition).
            wf = cpool.tile([P, KT, OUT], fp16)
            for r in range(wrep):
                for k in range(KT):
                    eng = nc.scalar if k % 2 == 0 else nc.sync
                    eng.dma_start(wf[:, k, :],
                                  wfh[:, k * OUT:(k + 1) * OUT])
            bias_sb = cpool.tile([P, NOB], f32)
            nc.scalar.dma_start(bias_sb[:], biasd[:, :])
            # Channel centers in h units: (c-1) - 1/h, exact in fp16 for h=.4
            gt = cpool.tile([P, NCH7, BCH], fp16)
            for c in range(NCH7):
                nc.vector.memset(gt[:, c, :], float(c - 1) - 1.0 / h)

            # ---- main loop ----
            first = True
            for bc in [c for _ in range(repeat) for c in range(NBC)]:
                bas = []
                for t in range(NT):
                    if first and bc == 0:
                        xc = pref[t]
                    else:
                        xc = xpool.tile([P, BCH], f32, tag=f"xc{t}")
                        nc.sync.dma_start(
                            xc[:], xt[t * P:(t + 1) * P,
                                      bc * BCH:(bc + 1) * BCH])
                    xh = xpool.tile([P, BCH], fp16, tag=f"xh{t}")
                    nc.vector.tensor_scalar(xh[:], xc[:], 1.0 / h, None,
                                            ALU.mult)
                    sh3 = [P, NCH7, BCH]
                    xb = xh[:].unsqueeze(1).broadcast_to(sh3)
                    t8 = tpool.tile(sh3, fp16, tag=f"t8{t}")
                    nc.vector.tensor_sub(t8[:], xb, gt[:])
                    # t8 := a = s2*|t|  (ACT abs with folded scale)
                    nc.scalar.activation(t8[:], t8[:], AF.Abs, scale=S2)
                    r2 = tpool.tile(sh3, fp16, tag=f"r2{t}")
                    nc.vector.tensor_scalar(r2[:], t8[:], -1.0, 2.0 * S2,
                                            ALU.mult, ALU.add)
                    nc.vector.tensor_scalar_max(r2[:], r2[:], 0.0)
                    v = tpool.tile(sh3, fp16, tag=f"v{t}")
                    nc.vector.tensor_scalar(v[:], t8[:], -S1 / S2, S1,
                                            ALU.mult, ALU.add)
                    nc.vector.tensor_scalar_max(v[:], v[:], 0.0)  # := r1
                    q2 = tpool.tile(sh3, fp16, tag=f"q2{t}")
                    nc.vector.tensor_mul(q2[:], r2[:], r2[:])
                    q1 = tpool.tile(sh3, fp16, tag=f"q1{t}")
                    nc.scalar.activation(q1[:], v[:], AF.Square)
                    nc.vector.tensor_mul(q2[:], q2[:], r2[:])  # := r2^3
                    nc.vector.tensor_mul(q1[:], q1[:], v[:])   # := r1^3
                    bt_ = bpool.tile(sh3, fp16, tag=f"bas{t}")
                    nc.gpsimd.tensor_sub(bt_[:], q2[:], q1[:])
                    bas.append(bt_)
                first = False

                for ob in range(NOB):
                    ps = ppool.tile([P, BCH], f32, tag="ps")
                    k = 0
                    for t in range(NT):
                        for c in range(NCH7):
                            nc.tensor.matmul(
                                ps[:],
                                wf[:, t * NCH7 + c, ob * P:(ob + 1) * P],
                                bas[t][:, c, :],
                                start=(k == 0), stop=(k == KT - 1))
                            k += 1
                    osb = opool.tile([P, BCH], f32, tag="osb")
                    nc.scalar.activation(osb[:], ps[:], AF.Identity,
                                         bias=bias_sb[:, ob:ob + 1],
                                         scale=1.0)
                    nc.sync.dma_start(
                        outp[ob * P:(ob + 1) * P,
                             bc * BCH:(bc + 1) * BCH], osb[:])

    nc.compile()
    return nc


def _stage(x, base_weight, spline_weight, grid):
    """Per-core staging: quantize + fold on host in f32 (bit-identical to
    the old on-device prologue: same RNE rounding, same op order), pack
    weights fp16."""
    h = np.float32(grid[0, 1] - grid[0, 0])
    gam7 = np.float32((NCH - 2) * h - 1.0)
    in_maps = []
    # round(32w)/32 in f32, RNE — matches the device magic-number round.
    q_all = (np.round(spline_weight.astype(np.float32) * np.float32(32.0))
             * np.float32(1.0 / 32.0)).astype(np.float32)
    for j in range(NCORES):
        sh = slice(j * ISH, (j + 1) * ISH)
        xt = np.ascontiguousarray(x[:, sh].T)
        q = q_all[:, sh, :]                       # [OUT, 256, 8] f32
        bw = base_weight[:, sh].astype(np.float32)  # [OUT, 256]
        q7 = q[:, :, NCH - 1]
        v7 = q7 + gam7 * bw                        # [OUT, 256]
        bias = v7.sum(axis=1, dtype=np.float32)    # [OUT]
        # folded channels c=0..6: (q_c - q_7) + (c-7)*h*bw
        wfold = np.empty((OUT, ISH, NCH7), dtype=np.float32)
        for c in range(NCH7):
            wfold[:, :, c] = (q[:, :, c] - q7) + np.float32((c - 7) * h) * bw
        # layout [p, t*7+c, o]: i = t*128+p
        wfh = wfold.reshape(OUT, NT, P, NCH7).transpose(2, 1, 3, 0)
        wfh = np.ascontiguousarray(
            wfh.reshape(P, KT * OUT).astype(np.float16))
        biasd = np.ascontiguousarray(bias.reshape(NOB, P).T)
        in_maps.append({"xt": xt, "wfh": wfh, "biasd": biasd})
    return in_maps


def kernel(x, base_weight, spline_weight, grid, _profile=None):
    from concourse import bass_utils

    x = np.asarray(x, dtype=np.float32)
    base_weight = np.asarray(base_weight, dtype=np.float32)
    spline_weight = np.asarray(spline_weight, dtype=np.float32)
    grid = np.asarray(grid, dtype=np.float32)

    h = float(grid[0, 1] - grid[0, 0])
    key = round(h, 9)
    if key not in _BUILT:
        _BUILT[key] = _build(h)
    nc = _BUILT[key]

    in_maps = _stage(x, base_weight, spline_weight, grid)
    kw = {}
    if _profile is not None:
        kw = _profile
    res = bass_utils.run_bass_kernel_spmd(
        nc, in_maps, core_ids=list(range(NCORES)), **kw)

    out_T = np.zeros((OUT, B), dtype=np.float32)
    for om in res.results:
        out_T += np.asarray(om["outp"], dtype=np.float32)
    if _profile is not None:
        kernel._last_result = res
    return np.ascontiguousarray(out_T.T)
